# revision 12
# baseline (speedup 1.0000x reference)
"""DeepSeekMoE layer (T=2048, D=1024, E=8 experts top-2, shared-expert I=2048)
as a Bass/Tile SPMD kernel on 8 Trainium2 NeuronCores.

Sharding (expert-parallel, per the module's own structure):
  - core c owns routed expert c (w1/w2/w3/b1/b2/b3 slice c)
  - shared-expert MLP inter dim (2048) split 8-way: core c owns rows
    [256c, 256(c+1)) of sw1/sw2 (column-parallel) and the matching columns
    of sw3 (row-parallel)
  - gate replicated (every core computes full softmax scores; it only keeps
    the mask/weight column of its own expert, passed as an extra gate column)
  - each core emits its partial sum y_c + z_c as a (1024, 2048) [d, t] tensor;
    the unshard step sums the partials host-side and transposes back.

Kernel structure per core:
  Phase 0 (gate): stream x^T (true fp32 copy) through 8 token segments,
    compute logits[t, 0:8] + own column in exact fp32 on the PE, do the
    softmax / top-2 on-chip, and write two row vectors to DRAM scratch:
    routing weight w[t] and mask m[t] for the own expert. One Exp table load.
  Phase 1 (experts): per 256-token segment, with all weights resident in
    f32r (full-rate fp32 storage, both operands RNE-rounded to 11-bit
    mantissa by the PE): shared-expert g/u/z matmuls, in-place xs = x*w
    scaling, routed h1/x3/x2, fused epilogue out = (x2+b2)*x3*m + z.
    All matmuls keep weights stationary so outputs are [row, token] and all
    per-row biases are ACT per-partition biases; per-token scalars (w, m)
    are partition-broadcast tiles re-read from the DRAM scratch.
"""

import os
import sys

for _p in ("/opt/trn_rl_repo", os.path.expanduser("~/.axon_site/_ro/trn_rl_repo")):
    if os.path.isdir(_p) and _p not in sys.path:
        sys.path.insert(0, _p)
        break

from contextlib import ExitStack

import numpy as np

import concourse.bass as bass
from concourse import bacc
import concourse.mybir as mybir
import concourse.tile as tile
from concourse.bass_utils import run_bass_kernel_spmd

F32 = mybir.dt.float32
F32R = mybir.dt.float32r
AF = mybir.ActivationFunctionType
OP = mybir.AluOpType

T = 2048      # tokens
D = 1024      # model dim
H = 1024      # expert hidden dim
E = 8         # routed experts
IS = 256      # shared-expert inter dim per core (2048 / 8)
IK = IS // 128
P = 128
DK = D // P   # 8 contraction chunks over d
HK = H // P   # 8 chunks over h
TSEG = 256    # token segment (matmul moving free dim; >=256 keeps f32r fast)
NSEG = T // TSEG
TM = TSEG // P
NCORES = 8

_NC_CACHE = {}


def build_module():
    nc = bacc.Bacc("TRN2", target_bir_lowering=False, debug=False)

    xTd = nc.dram_tensor("xT", [D, T], F32R, kind="ExternalInput")
    xTfd = nc.dram_tensor("xTf", [D, T], F32, kind="ExternalInput")
    g9d = nc.dram_tensor("gate9", [D, E + 1], F32, kind="ExternalInput")
    w1d = nc.dram_tensor("w1T", [D, H], F32R, kind="ExternalInput")
    w2d = nc.dram_tensor("w2T", [H, D], F32R, kind="ExternalInput")
    w3d = nc.dram_tensor("w3T", [D, H], F32R, kind="ExternalInput")
    b1d = nc.dram_tensor("b1c", [P, HK], F32, kind="ExternalInput")
    b2d = nc.dram_tensor("b2c", [P, DK], F32, kind="ExternalInput")
    b3d = nc.dram_tensor("b3c", [P, HK], F32, kind="ExternalInput")
    s1d = nc.dram_tensor("sw1sT", [D, IS], F32R, kind="ExternalInput")
    s2d = nc.dram_tensor("sw2sT", [D, IS], F32R, kind="ExternalInput")
    s3d = nc.dram_tensor("sw3sT", [IS, D], F32R, kind="ExternalInput")
    outd = nc.dram_tensor("out", [D, T], F32, kind="ExternalOutput")

    with tile.TileContext(nc) as tc:
        build_tile_kernel(
            tc, xTd, xTfd, g9d, w1d, w2d, w3d, b1d, b2d, b3d, s1d, s2d, s3d, outd
        )
    nc.compile()
    return nc


def build_tile_kernel(tc, xTd, xTfd, g9d, w1d, w2d, w3d, b1d, b2d, b3d, s1d, s2d, s3d, outd):
    nc = tc.nc
    ctx = ExitStack()
    resident = ctx.enter_context(tc.tile_pool(name="resident", bufs=1))
    xt_pool = ctx.enter_context(tc.tile_pool(name="xt", bufs=2))
    seg_pool = ctx.enter_context(tc.tile_pool(name="seg", bufs=1))
    out_pool = ctx.enter_context(tc.tile_pool(name="outp", bufs=2))
    bc_pool = ctx.enter_context(tc.tile_pool(name="bc", bufs=2))
    gsmall = ctx.enter_context(tc.tile_pool(name="gsmall", bufs=2))
    ps_mm = ctx.enter_context(tc.tile_pool(name="psmm", bufs=4, space="PSUM"))
    ps_g = ctx.enter_context(tc.tile_pool(name="psg", bufs=2, space="PSUM"))
    dram = ctx.enter_context(tc.tile_pool(name="dram", bufs=1, space="DRAM"))

    # ---- resident weights ----
    w1T = resident.tile([P, DK, H], F32R)
    nc.sync.dma_start(out=w1T, in_=w1d.ap().rearrange("(k p) h -> p k h", p=P))
    w2T = resident.tile([P, HK, D], F32R)
    nc.sync.dma_start(out=w2T, in_=w2d.ap().rearrange("(k p) h -> p k h", p=P))
    w3T = resident.tile([P, DK, H], F32R)
    nc.sync.dma_start(out=w3T, in_=w3d.ap().rearrange("(k p) h -> p k h", p=P))
    sw1sT = resident.tile([P, DK, IS], F32R)
    nc.sync.dma_start(out=sw1sT, in_=s1d.ap().rearrange("(k p) i -> p k i", p=P))
    sw2sT = resident.tile([P, DK, IS], F32R)
    nc.sync.dma_start(out=sw2sT, in_=s2d.ap().rearrange("(k p) i -> p k i", p=P))
    sw3sT = resident.tile([P, IK, D], F32R)
    nc.sync.dma_start(out=sw3sT, in_=s3d.ap().rearrange("(k p) d -> p k d", p=P))
    g9 = resident.tile([P, DK, E + 1], F32)
    nc.sync.dma_start(out=g9, in_=g9d.ap().rearrange("(k p) e -> p k e", p=P))
    b1c = resident.tile([P, HK], F32)
    nc.sync.dma_start(out=b1c, in_=b1d.ap())
    b2c = resident.tile([P, DK], F32)
    nc.sync.dma_start(out=b2c, in_=b2d.ap())
    b3c = resident.tile([P, HK], F32)
    nc.sync.dma_start(out=b3c, in_=b3d.ap())

    # DRAM scratch: row 0 = routing weight w[t], row 1 = mask m[t]
    wm_dram = dram.tile([2, T], F32)

    xT_ap = xTd.ap().rearrange("(k p) (s t) -> p k s t", p=P, t=TSEG)
    xTf_ap = xTfd.ap().rearrange("(k p) (s t) -> p k s t", p=P, t=TSEG)
    out_ap = outd.ap().rearrange("(k p) (s t) -> p k s t", p=P, t=TSEG)

    # ================= Phase 0: gate for all tokens =================
    for seg in range(NSEG):
        xtsf = xt_pool.tile([P, DK, TSEG], F32, tag="xtsf")
        nc.sync.dma_start(out=xtsf, in_=xTf_ap[:, :, seg, :])

        ps_gate = ps_g.tile([P, TM, E + 1], F32)
        for tm in range(TM):
            for dk in range(DK):
                nc.tensor.matmul(
                    ps_gate[:, tm, :],
                    xtsf[:, dk, bass.ts(tm, P)],
                    g9[:, dk, :],
                    start=(dk == 0),
                    stop=(dk == DK - 1),
                )

        wmcol = gsmall.tile([P, TM, 2], F32, tag="wmcol")
        for tm in range(TM):
            lg = ps_gate[:, tm, :]
            mx = gsmall.tile([P, 1], F32, tag="mx")
            nc.vector.tensor_reduce(
                out=mx, in_=lg[:, 0:E], op=OP.max, axis=mybir.AxisListType.X
            )
            nmx = gsmall.tile([P, 1], F32, tag="nmx")
            nc.vector.tensor_scalar_mul(nmx, mx, -1.0)
            el = gsmall.tile([P, E + 1], F32, tag="el")
            nc.scalar.activation(el, lg, AF.Exp, bias=nmx, scale=1.0)
            ssum = gsmall.tile([P, 1], F32, tag="ssum")
            nc.vector.tensor_reduce(
                out=ssum, in_=el[:, 0:E], op=OP.add, axis=mybir.AxisListType.X
            )
            # top-2 selection on raw logits (monotone-safe vs the exp LUT)
            iseq = gsmall.tile([P, E], F32, tag="iseq")
            nc.vector.tensor_scalar(
                out=iseq, in0=lg[:, 0:E], scalar1=mx, scalar2=None, op0=OP.is_ge
            )
            lg2 = gsmall.tile([P, E], F32, tag="lg2")
            nc.vector.scalar_tensor_tensor(
                out=lg2, in0=iseq, scalar=-1e30, in1=lg[:, 0:E],
                op0=OP.mult, op1=OP.add,
            )
            top2 = gsmall.tile([P, 1], F32, tag="top2")
            nc.vector.tensor_reduce(
                out=top2, in_=lg2, op=OP.max, axis=mybir.AxisListType.X
            )
            rs = gsmall.tile([P, 1], F32, tag="rs")
            nc.vector.reciprocal(out=rs, in_=ssum)
            nc.vector.tensor_tensor(
                out=wmcol[:, tm, 0:1], in0=el[:, E : E + 1], in1=rs, op=OP.mult
            )
            nc.vector.tensor_scalar(
                out=wmcol[:, tm, 1:2], in0=lg[:, E : E + 1], scalar1=top2,
                scalar2=None, op0=OP.is_ge,
            )
        for tm in range(TM):
            off = seg * TSEG + tm * P
            nc.sync.dma_start(
                out=wm_dram[:, off : off + P].rearrange("c p -> p c"),
                in_=wmcol[:, tm, :],
            )

    # ================= Phase 1: experts =================
    for seg in range(NSEG):
        xts = xt_pool.tile([P, DK, TSEG], F32R, tag="xts")
        nc.sync.dma_start(out=xts, in_=xT_ap[:, :, seg, :])
        wbmb = bc_pool.tile([P, 2, TSEG], F32, tag="wbmb")
        nc.sync.dma_start(
            out=wbmb,
            in_=wm_dram[:, seg * TSEG : (seg + 1) * TSEG].partition_broadcast(P),
        )

        # shared expert: g = silu(x @ sw1s^T), u = x @ sw2s^T, gu in [i, t]
        gu = seg_pool.tile([P, IK, TSEG], F32R, tag="gu")
        for ik in range(IK):
            ps_gg = ps_mm.tile([P, TSEG], F32, tag="mm")
            for dk in range(DK):
                nc.tensor.matmul(
                    ps_gg, sw1sT[:, dk, bass.ts(ik, P)], xts[:, dk, :],
                    start=(dk == 0), stop=(dk == DK - 1),
                )
            nc.scalar.activation(gu[:, ik, :], ps_gg, AF.Silu)
            ps_uu = ps_mm.tile([P, TSEG], F32, tag="mm")
            for dk in range(DK):
                nc.tensor.matmul(
                    ps_uu, sw2sT[:, dk, bass.ts(ik, P)], xts[:, dk, :],
                    start=(dk == 0), stop=(dk == DK - 1),
                )
            nc.vector.tensor_tensor(
                out=gu[:, ik, :], in0=gu[:, ik, :].bitcast(F32), in1=ps_uu,
                op=OP.mult,
            )

        # xs^T = x^T * w, in place (after g/u consumed unscaled x)
        for dk in range(DK):
            nc.vector.tensor_tensor(
                out=xts[:, dk, :],
                in0=xts[:, dk, :].bitcast(F32),
                in1=wbmb[:, 0, :],
                op=OP.mult,
            )

        # routed: h1 = silu(xs @ w1^T + b1), x3 = xs @ w3^T + b3, [h, t]
        h1 = seg_pool.tile([P, HK, TSEG], F32R, tag="h1")
        x3 = seg_pool.tile([P, HK, TSEG], F32, tag="x3")
        for hk in range(HK):
            ps_h = ps_mm.tile([P, TSEG], F32, tag="mm")
            for dk in range(DK):
                nc.tensor.matmul(
                    ps_h, w1T[:, dk, bass.ts(hk, P)], xts[:, dk, :],
                    start=(dk == 0), stop=(dk == DK - 1),
                )
            nc.scalar.activation(
                h1[:, hk, :], ps_h, AF.Silu, bias=b1c[:, hk : hk + 1], scale=1.0
            )
            ps_3 = ps_mm.tile([P, TSEG], F32, tag="mm")
            for dk in range(DK):
                nc.tensor.matmul(
                    ps_3, w3T[:, dk, bass.ts(hk, P)], xts[:, dk, :],
                    start=(dk == 0), stop=(dk == DK - 1),
                )
            nc.scalar.activation(
                x3[:, hk, :], ps_3, AF.Identity, bias=b3c[:, hk : hk + 1], scale=1.0
            )

        # x2 = h1 @ w2^T + b2; z = gu @ sw3s^T; out = (x2+b2)*x3*m + z
        outs = out_pool.tile([P, DK, TSEG], F32, tag="outs")
        for dk in range(DK):
            ps_2 = ps_mm.tile([P, TSEG], F32, tag="mm")
            for hk in range(HK):
                nc.tensor.matmul(
                    ps_2, w2T[:, hk, bass.ts(dk, P)], h1[:, hk, :],
                    start=(hk == 0), stop=(hk == HK - 1),
                )
            ps_z = ps_mm.tile([P, TSEG], F32, tag="mm")
            for ik in range(IK):
                nc.tensor.matmul(
                    ps_z, sw3sT[:, ik, bass.ts(dk, P)], gu[:, ik, :],
                    start=(ik == 0), stop=(ik == IK - 1),
                )
            pp = seg_pool.tile([P, TSEG], F32, tag="pp")
            nc.vector.scalar_tensor_tensor(
                out=pp, in0=ps_2, scalar=b2c[:, dk : dk + 1], in1=x3[:, dk, :],
                op0=OP.add, op1=OP.mult,
            )
            nc.vector.tensor_tensor(out=pp, in0=pp, in1=wbmb[:, 1, :], op=OP.mult)
            nc.vector.tensor_tensor(out=outs[:, dk, :], in0=pp, in1=ps_z, op=OP.add)
        nc.sync.dma_start(out=out_ap[:, :, seg, :], in_=outs)
    ctx.close()


def _prep_inputs(x, gate_w, w1, b1, w2, b2, w3, b3, sw1, sw2, sw3):
    xt = np.asarray(x, dtype=np.float32).reshape(T, D)
    xT = np.ascontiguousarray(xt.T)
    in_maps = []
    for c in range(NCORES):
        gate9 = np.concatenate(
            [np.asarray(gate_w, np.float32).T, np.asarray(gate_w[c], np.float32)[:, None]],
            axis=1,
        )
        in_maps.append(
            {
                "xT": xT,
                "xTf": xT,
                "gate9": np.ascontiguousarray(gate9),
                "w1T": np.ascontiguousarray(np.asarray(w1[c], np.float32).T),
                "w2T": np.ascontiguousarray(np.asarray(w2[c], np.float32).T),
                "w3T": np.ascontiguousarray(np.asarray(w3[c], np.float32).T),
                "b1c": np.ascontiguousarray(np.asarray(b1[c], np.float32).reshape(HK, P).T),
                "b2c": np.ascontiguousarray(np.asarray(b2[c], np.float32).reshape(DK, P).T),
                "b3c": np.ascontiguousarray(np.asarray(b3[c], np.float32).reshape(HK, P).T),
                "sw1sT": np.ascontiguousarray(np.asarray(sw1[c * IS : (c + 1) * IS], np.float32).T),
                "sw2sT": np.ascontiguousarray(np.asarray(sw2[c * IS : (c + 1) * IS], np.float32).T),
                "sw3sT": np.ascontiguousarray(np.asarray(sw3[:, c * IS : (c + 1) * IS], np.float32).T),
            }
        )
    return in_maps


def run(inputs_dict, trace=False, **kw):
    if "nc" not in _NC_CACHE:
        _NC_CACHE["nc"] = build_module()
    nc = _NC_CACHE["nc"]
    in_maps = _prep_inputs(**inputs_dict)
    res = run_bass_kernel_spmd(
        nc, in_maps, core_ids=list(range(NCORES)), trace=trace, **kw
    )
    acc = np.zeros((D, T), dtype=np.float64)
    for c in range(NCORES):
        acc += res.results[c]["out"].astype(np.float64)
    out = acc.T.reshape(1, T, D).astype(np.float32)
    return out, res


def kernel(**inputs):
    out, _ = run(inputs)
    return out
